# revision 8
# baseline (speedup 1.0000x reference)
"""Trainium2 Bass kernel for capsule routing (nn_Capsule).

Reference computation:
    u_hat = einsum('bic,ce->bie', u_vecs, W).reshape(B, I, N, D).transpose(0,2,1,3)
    b = 0
    for r in range(3):
        c = softmax(b, axis=1)                      # over capsules n
        out = squash(einsum('bni,bnid->bnd', c, u_hat))
        if r < 2: b = einsum('bnd,bnid->bni', out, u_hat)
    return out    # (B, N, D)

u_hat (32 MiB/core) is never materialized; every use factors through u and W:
    round 0:  c uniform = 1/N  ->  out0 = squash((1/N) * (sum_i u[b,i,:]) @ W)
    V[b,c,n]     = sum_d W[c,(n,d)] o[b,n,d]
    logits[b,i,n]= sum_c u[b,i,c] V[b,c,n]
    T[b,n,c]     = sum_i softmax(logits)[b,i,n] * u[b,i,c]
    pre[b,n,d]   = sum_c T[b,n,c] W[c,(n,d)]   -> out = squash(pre)

Implementation notes (vs a straightforward per-(b,n) loop):
  - all inputs are cast to bf16 on the host, halving HBM traffic and making
    every matmul stationary a full 128x128 bf16 tile (fast-weight-load path),
  - u^T and W^T are produced by the DMA xbar transpose straight from DRAM
    (no PE transposes, no PSUM->SBUF copy traffic in setup),
  - V is computed with a zero-padded block-diagonal moving operand O
    ([nd, k, (b,n)], zeros everywhere except a per-round diagonal holding o),
    so each (ck, k) matmul writes its own disjoint 8 columns of one PSUM tile,
  - pre is computed per nd-chunk k (capsule pair 2k, 2k+1) with the matching
    8 (b, n-pair) columns of T^T as the moving operand; only the matching
    row-half of each output column is valid and downstream reads honor that,
  - squash reductions over d (a partition-dim reduction) use tiny block-ones
    matmuls ([128,2] / [2,128]) to sum and re-broadcast per 64-row half;
    rsqrt is a bit-trick seed + 2 Newton steps on the Vector engine so the
    Scalar engine's activation table stays pinned on exp,
  - a burst of dummy matmuls on a constant tile during the DMA window keeps
    the PE clock ramped before the real work arrives.

Sharding: data-parallel over batch, 4 batches per core x 8 cores, W replicated.
"""

import numpy as np
from contextlib import ExitStack

import ml_dtypes

import concourse.bass as bass
import concourse.bacc as bacc
import concourse.tile as tile
from concourse import mybir
from concourse.bass_utils import run_bass_kernel_spmd

B, I, C = 32, 1024, 256
N, D = 32, 64
ND = N * D
ROUTINGS = 3
EPS = 1e-7
NCORES = 8
BL = B // NCORES   # batches per core
IC = I // 128      # i chunks of 128
CK = C // 128      # c chunks of 128
NDK = ND // 128    # (n,d) chunks of 128
F32 = mybir.dt.float32
U32 = mybir.dt.uint32
BF16 = mybir.dt.bfloat16
MULT = mybir.AluOpType.mult
AF = mybir.ActivationFunctionType
RSQRT_MAGIC = 0x5F3759DF
WARMUP_MM = 48


def _capsule_body(ctx: ExitStack, tc: tile.TileContext, out_ap, u_ap, w_ap):
    nc = tc.nc
    ctx.enter_context(nc.allow_low_precision(reason="bf16 pipeline"))

    const = ctx.enter_context(tc.tile_pool(name="const", bufs=1))
    persist = ctx.enter_context(tc.tile_pool(name="persist", bufs=1))
    work = ctx.enter_context(tc.tile_pool(name="work", bufs=2))

    # ---- constants ----
    # blk1[:, e] = 1 on partition half e (column sums per 64-row half)
    blk1 = const.tile([128, 2], BF16)
    nc.gpsimd.memset(blk1[:], 0.0)
    nc.gpsimd.memset(blk1[0:64, 0:1], 1.0)
    nc.gpsimd.memset(blk1[64:128, 1:2], 1.0)
    # blk1T[e, p] = 1 where p//64 == e (re-broadcast per half); _n variant
    # folds the uniform-routing 1/N of round 0. Engine writes can't start at
    # partition 1, so the two-row constants take a DRAM round-trip: stage the
    # pattern flat on partition 0, DMA out and back as [2, 2, 128].
    cstage = const.tile([1, 512], F32)
    nc.gpsimd.memset(cstage[:], 0.0)
    nc.gpsimd.memset(cstage[:, 0:64], 1.0)
    nc.gpsimd.memset(cstage[:, 128:192], 1.0 / N)
    nc.gpsimd.memset(cstage[:, 320:384], 1.0)
    nc.gpsimd.memset(cstage[:, 448:512], 1.0 / N)
    cdram = nc.dram_tensor("cscratch", [512], F32, kind="Internal").ap()
    nc.sync.dma_start(out=cdram, in_=cstage[:])
    cb = const.tile([2, 2, 128], F32)
    nc.sync.dma_start(out=cb[:], in_=cdram.rearrange("(p j c) -> p j c", p=2, j=2))
    blk1T = cb[:, 0]
    blk1T_n = cb[:, 1]
    magic = const.tile([2, 128], U32)
    nc.gpsimd.memset(magic[:], RSQRT_MAGIC)
    warm = const.tile([128, 128], BF16)
    nc.gpsimd.memset(warm[:], 0.001)

    # ---- persistent SBUF tensors ----
    u_bf = persist.tile([128, IC, BL, C], BF16)   # [i, ic, b, c]
    ut = persist.tile([128, CK, BL, I], BF16)     # [c, ck, b, i]
    w_sb = persist.tile([128, CK, ND], BF16)      # [c, ck, (n,d)]
    wt2 = persist.tile([128, NDK, C], BF16)       # [(n,d), k, c]
    o_diag = persist.tile([128, NDK, 128], BF16)  # [(n,d), k, (b,n)] zeros + diag
    csm = persist.tile([128, IC, 128], BF16)      # [i, ic, (b,n)]
    st = persist.tile([128, CK, BL], F32)         # [c, ck, b] col sums of u
    st8 = persist.tile([128, CK, 8], BF16)        # st widened to (b, e) cols
    oT2 = persist.tile([128, NDK, 8], BF16)       # squash output feeding o_diag
    oF = persist.tile([128, NDK, 8], F32)         # final-round fp32 output

    nc.gpsimd.memset(o_diag[:], 0.0)

    # ---- input DMA (sync queue) + xbar transposes (scalar queue) ----
    nc.sync.dma_start(
        out=w_sb[:], in_=w_ap.rearrange("(ck p) nd -> p ck nd", p=128)
    )
    nc.sync.dma_start_transpose(wt2[:], w_ap)  # [C, ND] -> [128, NDK, C]
    u_r = u_ap.rearrange("b (ic p) c -> b p ic c", p=128)
    for b in range(BL):
        nc.sync.dma_start(out=u_bf[:, :, b, :], in_=u_r[b])
    u_flat = u_ap.rearrange("b i c -> (b i) c")
    for ck in range(CK):
        nc.scalar.dma_start_transpose(
            ut[:, ck], u_flat[:, ck * 128:(ck + 1) * 128]
        )

    # preload the exp activation table while DMAs are in flight
    warm_act = work.tile([1, 2], F32, tag="wa")
    nc.scalar.activation(out=warm_act[:], in_=blk1T[0:1, 0:2], func=AF.Exp)

    ps_warm = ctx.enter_context(tc.tile_pool(name="ps_warm", bufs=1, space="PSUM"))
    ps_v = ctx.enter_context(tc.tile_pool(name="ps_v", bufs=1, space="PSUM"))
    ps_lg = ctx.enter_context(tc.tile_pool(name="ps_lg", bufs=1, space="PSUM"))
    ps_t = ctx.enter_context(tc.tile_pool(name="ps_t", bufs=1, space="PSUM"))
    ps_pre = ctx.enter_context(tc.tile_pool(name="ps_pre", bufs=1, space="PSUM"))
    ps_sq = ctx.enter_context(tc.tile_pool(name="ps_sq", bufs=1, space="PSUM"))

    # PE warmup: ramp the tensor-engine clock during the DMA window
    warm_ps = ps_warm.tile([128, 128], F32, tag="warm")
    with nc.named_scope("warmup"):
        for _ in range(WARMUP_MM):
            nc.tensor.matmul(
                out=warm_ps[:], lhsT=warm[:], rhs=warm[:], start=True, stop=True
            )

    # ---- st: column sums of u over i (from ut), split across DVE and ACT ----
    with nc.named_scope("st"):
        scratch = work.tile([128, I], BF16, tag="stscratch")
        for b in range(BL):
            for ck in range(CK):
                if (b + ck) % 2 == 0:
                    nc.vector.reduce_sum(
                        out=st[:, ck, b:b + 1], in_=ut[:, ck, b, :],
                        axis=mybir.AxisListType.X,
                    )
                else:
                    nc.scalar.activation(
                        out=scratch[:], in_=ut[:, ck, b, :], func=AF.Copy,
                        accum_out=st[:, ck, b:b + 1],
                    )
        st8v = st8[:].rearrange("p h (b e) -> p h b e", e=2)
        for e in range(2):
            nc.vector.tensor_copy(out=st8v[:, :, :, e], in_=st[:])

    def emit_pre(r, tt_sb):
        """pre_ps[p, k, (b,e)] = sum_c W[c, k-chunk] . moving; valid rows of
        column (b, e) are the half p//64 == e."""
        pre_ps = ps_pre.tile([128, NDK, 8], F32, tag="pre")
        with nc.named_scope(f"r{r}_pre"):
            for k in range(NDK):
                for h in range(CK):
                    if r == 0:
                        rhs = st8[:, h, :]
                    else:
                        rhs = tt_sb[:, h, :, 2 * k:2 * k + 2]
                    nc.tensor.matmul(
                        out=pre_ps[:, k, :],
                        lhsT=w_sb[:, h, k * 128:(k + 1) * 128],
                        rhs=rhs,
                        start=(h == 0),
                        stop=(h == CK - 1),
                    )
        return pre_ps

    def emit_squash(r, pre_ps):
        """o = pre * rsqrt(blocksum(pre^2) * scale + eps); writes oT2 (bf16,
        rounds 0/1) or oF (fp32, final round)."""
        with nc.named_scope(f"r{r}_sq"):
            sq_sb = work.tile([128, NDK, 8], BF16, tag="sq")
            nc.scalar.activation(out=sq_sb[:], in_=pre_ps[:], func=AF.Square)
            pre_sb = work.tile([128, NDK, 8], F32, tag="presb")
            nc.vector.tensor_copy(out=pre_sb[:], in_=pre_ps[:])
            ss_ps = ps_sq.tile([2, 128], F32, tag="ss")
            nc.tensor.matmul(
                out=ss_ps[:], lhsT=blk1[:],
                rhs=sq_sb[:].rearrange("p a b -> p (a b)"),
                start=True, stop=True,
            )
            x_sb = work.tile([2, 128], F32, tag="x")
            nc.vector.tensor_scalar(
                out=x_sb[:], in0=ss_ps[:],
                scalar1=(1.0 / (N * N) if r == 0 else 1.0), scalar2=EPS,
                op0=MULT, op1=mybir.AluOpType.add,
            )
            # rsqrt: y0 = bitcast(magic - (bitcast(x) >> 1)); 2 Newton steps
            yb_sb = work.tile([2, 128], U32, tag="yb")
            nc.vector.tensor_scalar(
                out=yb_sb[:], in0=x_sb[:].bitcast(U32), scalar1=1, scalar2=None,
                op0=mybir.AluOpType.logical_shift_right,
            )
            y_sb = work.tile([2, 128], F32, tag="y")
            nc.vector.tensor_tensor(
                y_sb[:].bitcast(U32), magic[:], yb_sb[:],
                mybir.AluOpType.subtract,
            )
            for _ in range(2):
                t1 = work.tile([2, 128], F32, tag="nt1")
                nc.vector.tensor_mul(t1[:], y_sb[:], y_sb[:])
                nc.vector.scalar_tensor_tensor(
                    out=t1[:], in0=t1[:], scalar=-0.5, in1=x_sb[:],
                    op0=MULT, op1=MULT,
                )
                y2 = work.tile([2, 128], F32, tag="y2")
                nc.vector.scalar_tensor_tensor(
                    out=y2[:], in0=t1[:], scalar=1.5, in1=y_sb[:],
                    op0=mybir.AluOpType.add, op1=MULT,
                )
                y_sb = y2
            rsqE_ps = ps_sq.tile([128, 128], F32, tag="rsqE")
            nc.tensor.matmul(
                out=rsqE_ps[:], lhsT=(blk1T_n[:] if r == 0 else blk1T[:]),
                rhs=y_sb[:], start=True, stop=True,
            )
            dst = oF if r == ROUTINGS - 1 else oT2
            nc.vector.tensor_tensor(
                dst[:].rearrange("p a b -> p (a b)"), pre_sb[:].rearrange("p a b -> p (a b)"),
                rsqE_ps[:], MULT,
            )

    # ---- round 0 ----
    pre_ps = emit_pre(0, None)
    emit_squash(0, pre_ps)

    o_pstride = o_diag[:].ap[0][0]
    for r in range(1, ROUTINGS):
        # o_diag update: O[e*64+d, k, 32b+2k+e] = oT2[e*64+d, k, 2b+e]
        with nc.named_scope(f"r{r}_od"):
            for b in range(BL):
                for e in range(2):
                    base = o_diag[e * 64:e * 64 + 1, 0:1, (32 * b + e):(32 * b + e + 1)]
                    dstap = bass.AP(
                        tensor=base.tensor, offset=base.offset,
                        ap=[[o_pstride, 64], [130, NDK]],
                    )
                    nc.vector.tensor_copy(
                        out=dstap,
                        in_=oT2[e * 64:(e + 1) * 64, :, 2 * b + e],
                    )
        # V[c, ck, (b,n)]: per (ck, k) one matmul into disjoint 8 columns
        v_ps = ps_v.tile([128, CK, 4, 32], F32, tag="v")
        with nc.named_scope(f"r{r}_v"):
            ov = o_diag[:].rearrange("p k (b n) -> p k b n", b=BL)
            for ck in range(CK):
                for k in range(NDK):
                    nc.tensor.matmul(
                        out=v_ps[:, ck, :, 2 * k:2 * k + 2],
                        lhsT=wt2[:, k, ck * 128:(ck + 1) * 128],
                        rhs=ov[:, k, :, 2 * k:2 * k + 2],
                        start=True, stop=True,
                    )
            v_sb = work.tile([128, CK, 128], BF16, tag="vsb")
            nc.vector.tensor_copy(
                out=v_sb[:], in_=v_ps[:].rearrange("p c b n -> p c (b n)")
            )
        # logits[i, b, ic, n] then per-b softmax over n
        lg_ps = ps_lg.tile([128, BL, IC, N], F32, tag="lg")
        with nc.named_scope(f"r{r}_lg"):
            for b in range(BL):
                for ic in range(IC):
                    for ck in range(CK):
                        nc.tensor.matmul(
                            out=lg_ps[:, b, ic, :],
                            lhsT=ut[:, ck, b, ic * 128:(ic + 1) * 128],
                            rhs=v_sb[:, ck, 32 * b:32 * b + 32],
                            start=(ck == 0),
                            stop=(ck == CK - 1),
                        )
        with nc.named_scope(f"r{r}_sm"):
            for b in range(BL):
                e_sb = work.tile([128, IC, N], BF16, tag="e")
                nc.scalar.activation(out=e_sb[:], in_=lg_ps[:, b], func=AF.Exp)
                s_sb = work.tile([128, IC], F32, tag="s")
                nc.vector.reduce_sum(
                    out=s_sb[:], in_=e_sb[:], axis=mybir.AxisListType.X
                )
                sr_sb = work.tile([128, IC], F32, tag="sr")
                nc.vector.reciprocal(out=sr_sb[:], in_=s_sb[:])
                nc.vector.tensor_tensor(
                    csm[:, :, 32 * b:32 * b + 32],
                    e_sb[:],
                    sr_sb[:, :, None].to_broadcast([128, IC, N]),
                    MULT,
                )
        # T^T[c, ck, b, n] = sum_i u[i, c] csm[i, (b,n)]
        tt_ps = ps_t.tile([128, CK, BL, N], F32, tag="tt")
        with nc.named_scope(f"r{r}_t"):
            for b in range(BL):
                for ck in range(CK):
                    for ic in range(IC):
                        nc.tensor.matmul(
                            out=tt_ps[:, ck, b, :],
                            lhsT=u_bf[:, ic, b, ck * 128:(ck + 1) * 128],
                            rhs=csm[:, ic, 32 * b:32 * b + 32],
                            start=(ic == 0),
                            stop=(ic == IC - 1),
                        )
            tt_sb = work.tile([128, CK, BL, N], BF16, tag="ttsb")
            nc.vector.tensor_copy(out=tt_sb[:], in_=tt_ps[:])
        pre_ps = emit_pre(r, tt_sb)
        emit_squash(r, pre_ps)

    # ---- write out: out[b, 2k+e, d] = oF[e*64+d, k, 2b+e] ----
    with nc.named_scope("out"):
        for e in range(2):
            for b in range(BL):
                dst = bass.AP(
                    tensor=out_ap.tensor,
                    offset=out_ap.offset + b * N * D + e * D,
                    ap=[[1, D], [2 * D, NDK]],
                )
                nc.sync.dma_start(
                    out=dst, in_=oF[e * 64:(e + 1) * 64, :, 2 * b + e]
                )


def build_program():
    nc = bacc.Bacc("TRN2", target_bir_lowering=False, debug=False)
    u_ap = nc.dram_tensor("u", [BL, I, C], BF16, kind="ExternalInput").ap()
    w_ap = nc.dram_tensor("w", [C, ND], BF16, kind="ExternalInput").ap()
    out_ap = nc.dram_tensor("out", [BL, N, D], F32, kind="ExternalOutput").ap()
    with tile.TileContext(nc) as tc:
        with ExitStack() as ctx:
            _capsule_body(ctx, tc, out_ap, u_ap, w_ap)
    nc.compile()
    return nc


def make_in_maps(u32: np.ndarray, w32: np.ndarray):
    u = np.ascontiguousarray(u32.astype(ml_dtypes.bfloat16))
    w = np.ascontiguousarray(w32.astype(ml_dtypes.bfloat16))
    return [{"u": u[i * BL:(i + 1) * BL], "w": w} for i in range(NCORES)]


_NC = None


def kernel(u_vecs: np.ndarray, W: np.ndarray) -> np.ndarray:
    global _NC
    u = np.asarray(u_vecs, dtype=np.float32)
    w = np.asarray(W, dtype=np.float32)
    assert u.shape == (B, I, C) and w.shape == (C, ND)
    if _NC is None:
        _NC = build_program()
    res = run_bass_kernel_spmd(_NC, make_in_maps(u, w), list(range(NCORES)))
    return np.concatenate(
        [res.results[i]["out"] for i in range(NCORES)], axis=0
    )


# revision 10
# speedup vs baseline: 1.1412x; 1.1412x over previous
"""Trainium2 Bass kernel for capsule routing (nn_Capsule).

Reference computation:
    u_hat = einsum('bic,ce->bie', u_vecs, W).reshape(B, I, N, D).transpose(0,2,1,3)
    b = 0
    for r in range(3):
        c = softmax(b, axis=1)                      # over capsules n
        out = squash(einsum('bni,bnid->bnd', c, u_hat))
        if r < 2: b = einsum('bnd,bnid->bni', out, u_hat)
    return out    # (B, N, D)

u_hat (32 MiB/core) is never materialized; every use factors through u and W:
    round 0:  c uniform = 1/N  ->  out0 = squash((1/N) * (sum_i u[b,i,:]) @ W)
    V[b,c,n]     = sum_d W[c,(n,d)] o[b,n,d]
    logits[b,i,n]= sum_c u[b,i,c] V[b,c,n]
    T[b,n,c]     = sum_i softmax(logits)[b,i,n] * u[b,i,c]
    pre[b,n,d]   = sum_c T[b,n,c] W[c,(n,d)]   -> out = squash(pre)

Implementation notes:
  - inputs are cast to bf16 on the host, halving HBM traffic and making every
    matmul stationary a full 128x128 bf16 tile (fast-weight-load path),
  - u^T comes from the DMA xbar transpose straight from DRAM (scalar queue)
    while u / W load on the sync and gpsimd queues — no PE setup transposes,
  - pre^T is computed per nd-chunk k (capsule pair 2k, 2k+1) with the 8
    matching (b, n-pair) columns of T^T moving; only the row-half matching a
    column's parity is valid, and the squash scale is built MASKED (zero on
    the invalid half) so o^T's garbage half reads as exact zeros,
  - that zero structure lets the next round's V contract o^T against W^T
    per nd-chunk directly: each (ck, k) matmul writes its own disjoint 8
    columns of one PSUM tile, no block-diagonal operand is materialized,
  - squash's partition-dim reductions over d use tiny block-ones matmuls
    ([128,2] / [2,128]) to sum and re-broadcast per 64-row half; rsqrt is a
    bit-trick seed + Newton steps on the Vector engine so the Scalar engine's
    activation table stays pinned on exp,
  - a burst of dummy matmuls on a constant tile during the DMA window keeps
    the PE clock ramped before the real work arrives.

Sharding: data-parallel over batch, 4 batches per core x 8 cores, W replicated.
"""

import numpy as np
from contextlib import ExitStack

import ml_dtypes

import concourse.bass as bass
import concourse.bacc as bacc
import concourse.tile as tile
from concourse import mybir
from concourse.bass_utils import run_bass_kernel_spmd
from concourse.masks import make_identity

B, I, C = 32, 1024, 256
N, D = 32, 64
ND = N * D
ROUTINGS = 3
EPS = 1e-7
NCORES = 8
BL = B // NCORES   # batches per core
IC = I // 128      # i chunks of 128
CK = C // 128      # c chunks of 128
NDK = ND // 128    # (n,d) chunks of 128
F32 = mybir.dt.float32
U32 = mybir.dt.uint32
BF16 = mybir.dt.bfloat16
MULT = mybir.AluOpType.mult
AF = mybir.ActivationFunctionType
RSQRT_MAGIC = 0x5F3759DF
WARMUP_MM = 40


def _capsule_body(ctx: ExitStack, tc: tile.TileContext, out_ap, u_ap, w_ap):
    nc = tc.nc
    ctx.enter_context(nc.allow_low_precision(reason="bf16 pipeline"))

    const = ctx.enter_context(tc.tile_pool(name="const", bufs=1))
    persist = ctx.enter_context(tc.tile_pool(name="persist", bufs=1))
    work = ctx.enter_context(tc.tile_pool(name="work", bufs=2))

    # ---- persistent SBUF tensors ----
    u_bf = persist.tile([128, IC, BL, C], BF16)   # [i, ic, b, c]
    ut = persist.tile([128, CK, BL, I], BF16)     # [c, ck, b, i]
    w_sb = persist.tile([128, CK, ND], BF16)      # [c, ck, (n,d)]
    wt2 = persist.tile([128, NDK, C], BF16)       # [(n,d), k, c]
    csm = persist.tile([128, IC, 128], BF16)      # [i, ic, (b,n)]
    st = persist.tile([128, CK, BL], F32)         # [c, ck, b] col sums of u
    st8 = persist.tile([128, CK, 8], BF16)        # st widened to (b, e) cols
    oT2 = persist.tile([128, NDK, 8], BF16)       # o^T, invalid half-zeros
    oF = persist.tile([128, NDK, 8], F32)         # final-round fp32 o^T
    o_out = persist.tile([128, BL, NDK], F32)     # DMA-friendly output layout

    # ---- input DMA: u^T via xbar on scalar, u on gpsimd, W on sync ----
    u_flat = u_ap.rearrange("b i c -> (b i) c")
    for ck in range(CK):
        nc.scalar.dma_start_transpose(
            ut[:, ck], u_flat[:, ck * 128:(ck + 1) * 128]
        )
    nc.sync.dma_start(
        out=w_sb[:], in_=w_ap.rearrange("(ck p) nd -> p ck nd", p=128)
    )
    u_r = u_ap.rearrange("b (ic p) c -> b p ic c", p=128)
    for b in range(BL):
        nc.gpsimd.dma_start(out=u_bf[:, :, b, :], in_=u_r[b])

    # ---- constants (all partition-0/64-aligned writes) ----
    ident = const.tile([128, 128], F32)
    make_identity(nc, ident[:])
    blk1 = const.tile([128, 2], BF16)
    nc.gpsimd.memset(blk1[:], 0.0)
    nc.gpsimd.memset(blk1[0:64, 0:1], 1.0)
    nc.gpsimd.memset(blk1[64:128, 1:2], 1.0)
    blk1f = const.tile([128, 2], F32)
    nc.gpsimd.memset(blk1f[:], 0.0)
    nc.gpsimd.memset(blk1f[0:64, 0:1], 1.0)
    nc.gpsimd.memset(blk1f[64:128, 1:2], 1.0)
    # bmask[p, j] = 1/64 where j%2 == p//64 -> M8 = blk1f^T @ bmask [2, 8]
    bmask = const.tile([128, 8], F32)
    nc.gpsimd.memset(bmask[:], 0.0)
    nc.gpsimd.memset(bmask[0:64, 0::2], 1.0 / 64)
    nc.gpsimd.memset(bmask[64:128, 1::2], 1.0 / 64)
    magic = const.tile([2, 128], U32)
    nc.gpsimd.memset(magic[:], RSQRT_MAGIC)
    warm = const.tile([128, 128], BF16)
    nc.gpsimd.memset(warm[:], 0.001)
    ident_bf = const.tile([128, 128], BF16)
    make_identity(nc, ident_bf[:])

    blk1T = const.tile([2, 128], F32)
    blk1T_n = const.tile([2, 128], F32)
    m8 = const.tile([2, 8], F32)

    with tc.tile_pool(name="ps_setup", bufs=2, space="PSUM") as ps_setup:
        # blk1T[e, p] = (p//64 == e) via PE transpose; M8[e, j] = (j%2 == e)
        cps = ps_setup.tile([2, 136], F32, tag="cps")
        nc.tensor.transpose(cps[:, 0:128], blk1f[:], ident[:])
        nc.tensor.matmul(
            out=cps[:, 128:136], lhsT=blk1f[:], rhs=bmask[:], start=True, stop=True
        )
        nc.vector.tensor_copy(out=blk1T[:], in_=cps[:, 0:128])
        nc.vector.tensor_scalar(
            out=blk1T_n[:], in0=cps[:, 0:128], scalar1=1.0 / N, scalar2=None,
            op0=MULT,
        )
        nc.vector.tensor_copy(out=m8[:], in_=cps[:, 128:136])

        # preload the exp activation table while DMAs are in flight
        warm_act = work.tile([1, 2], F32, tag="wa")
        nc.scalar.activation(out=warm_act[:], in_=blk1f[0:1, 0:2], func=AF.Exp)

        # PE warmup: ramp the tensor-engine clock during the DMA window
        warm_ps = ps_setup.tile([128, 128], F32, tag="warm")
        with nc.named_scope("warmup"):
            for _ in range(WARMUP_MM):
                nc.tensor.matmul(
                    out=warm_ps[:], lhsT=warm[:], rhs=warm[:],
                    start=True, stop=True,
                )

        # wt2[(nd), k, c] from w_sb via matmul-transpose (w_chunk^T @ I)
        with nc.named_scope("wtr"):
            for g in range(8):
                wt_ps = ps_setup.tile([128, 4, 128], F32, tag="wtr")
                for t in range(4):
                    h, k = divmod(g * 4 + t, NDK)
                    nc.tensor.matmul(
                        out=wt_ps[:, t, :],
                        lhsT=w_sb[:, h, k * 128:(k + 1) * 128],
                        rhs=ident_bf[:],
                        start=True, stop=True,
                    )
                h, k0 = divmod(g * 4, NDK)
                dstv = wt2[:, k0:k0 + 4, h * 128:(h + 1) * 128]
                if g % 2 == 0:
                    nc.vector.tensor_copy(out=dstv, in_=wt_ps[:])
                else:
                    nc.scalar.copy(out=dstv, in_=wt_ps[:])

    ps_v = ctx.enter_context(tc.tile_pool(name="ps_v", bufs=1, space="PSUM"))
    ps_lg = ctx.enter_context(tc.tile_pool(name="ps_lg", bufs=1, space="PSUM"))
    ps_t = ctx.enter_context(tc.tile_pool(name="ps_t", bufs=1, space="PSUM"))
    ps_pre = ctx.enter_context(tc.tile_pool(name="ps_pre", bufs=1, space="PSUM"))
    ps_sq = ctx.enter_context(tc.tile_pool(name="ps_sq", bufs=1, space="PSUM"))

    # ---- st: column sums of u over i (from ut), split across DVE and ACT ----
    with nc.named_scope("st"):
        scratch = work.tile([128, I], BF16, tag="stscratch")
        for ck in range(CK):
            for b in range(BL):
                if b % 2 == 0:
                    nc.vector.reduce_sum(
                        out=st[:, ck, b:b + 1], in_=ut[:, ck, b, :],
                        axis=mybir.AxisListType.X,
                    )
                else:
                    nc.scalar.activation(
                        out=scratch[:], in_=ut[:, ck, b, :], func=AF.Copy,
                        accum_out=st[:, ck, b:b + 1],
                    )
        st8v = st8[:].rearrange("p h (b e) -> p h b e", e=2)
        for e in range(2):
            nc.vector.tensor_copy(out=st8v[:, :, :, e], in_=st[:])

    def emit_pre(r, tt_sb):
        """pre_ps[p, k, (b,e)] = sum_c W[c, k-chunk] . moving; valid rows of
        column j=(b,e) are the half p//64 == e."""
        pre_ps = ps_pre.tile([128, NDK, 8], F32, tag="pre")
        with nc.named_scope(f"r{r}_pre"):
            for k in range(NDK):
                for h in range(CK):
                    if r == 0:
                        rhs = st8[:, h, :]
                    else:
                        rhs = tt_sb[:, h, :, 2 * k:2 * k + 2]
                    nc.tensor.matmul(
                        out=pre_ps[:, k, :],
                        lhsT=w_sb[:, h, k * 128:(k + 1) * 128],
                        rhs=rhs,
                        start=(h == 0),
                        stop=(h == CK - 1),
                    )
        return pre_ps

    def emit_squash(r, pre_ps):
        """o = pre * rsqrt(blocksum(pre^2) * scale + eps), with the rsqrt
        broadcast MASKED so each column's invalid row-half becomes zero."""
        with nc.named_scope(f"r{r}_sq"):
            sq_sb = work.tile([128, NDK, 8], BF16, tag="sq")
            nc.scalar.activation(out=sq_sb[:], in_=pre_ps[:], func=AF.Square)
            pre_sb = work.tile([128, NDK, 8], F32, tag="presb")
            nc.vector.tensor_copy(out=pre_sb[:], in_=pre_ps[:])
            ss_ps = ps_sq.tile([2, 128], F32, tag="ss")
            nc.tensor.matmul(
                out=ss_ps[:], lhsT=blk1[:],
                rhs=sq_sb[:].rearrange("p a b -> p (a b)"),
                start=True, stop=True,
            )
            x_sb = work.tile([2, 128], F32, tag="x")
            nc.vector.tensor_scalar(
                out=x_sb[:], in0=ss_ps[:],
                scalar1=(1.0 / (N * N) if r == 0 else 1.0), scalar2=EPS,
                op0=MULT, op1=mybir.AluOpType.add,
            )
            # rsqrt: y0 = bitcast(magic - (bitcast(x) >> 1)); 2 Newton steps
            yb_sb = work.tile([2, 128], U32, tag="yb")
            nc.vector.tensor_scalar(
                out=yb_sb[:], in0=x_sb[:].bitcast(U32), scalar1=1, scalar2=None,
                op0=mybir.AluOpType.logical_shift_right,
            )
            y_sb = work.tile([2, 128], F32, tag="y")
            nc.vector.tensor_tensor(
                y_sb[:].bitcast(U32), magic[:], yb_sb[:],
                mybir.AluOpType.subtract,
            )
            for _ in range(2):
                t1 = work.tile([2, 128], F32, tag="nt1")
                nc.vector.tensor_mul(t1[:], y_sb[:], y_sb[:])
                nc.vector.scalar_tensor_tensor(
                    out=t1[:], in0=t1[:], scalar=-0.5, in1=x_sb[:],
                    op0=MULT, op1=MULT,
                )
                y2 = work.tile([2, 128], F32, tag="y2")
                nc.vector.scalar_tensor_tensor(
                    out=y2[:], in0=t1[:], scalar=1.5, in1=y_sb[:],
                    op0=mybir.AluOpType.add, op1=MULT,
                )
                y_sb = y2
            # mask invalid (j-parity != row-half) entries to zero, then
            # re-broadcast per half: rsqE[p, (k,j)] = yM[p//64, (k,j)]
            ym = work.tile([2, NDK, 8], F32, tag="ym")
            nc.vector.tensor_tensor(
                ym[:], y_sb[:].rearrange("p (k j) -> p k j", j=8),
                m8[:, None, :].to_broadcast([2, NDK, 8]), MULT,
            )
            rsqE_ps = ps_sq.tile([128, 128], F32, tag="rsqE")
            nc.tensor.matmul(
                out=rsqE_ps[:], lhsT=(blk1T_n[:] if r == 0 else blk1T[:]),
                rhs=ym[:].rearrange("p a b -> p (a b)"), start=True, stop=True,
            )
            dst = oF if r == ROUTINGS - 1 else oT2
            nc.vector.tensor_tensor(
                dst[:].rearrange("p a b -> p (a b)"),
                pre_sb[:].rearrange("p a b -> p (a b)"),
                rsqE_ps[:], MULT,
            )

    # ---- round 0 ----
    pre_ps = emit_pre(0, None)
    emit_squash(0, pre_ps)

    for r in range(1, ROUTINGS):
        # V[c, ck, (b,n)]: per (ck, k) one matmul into disjoint 8 columns;
        # oT2's invalid halves are zero, so each k contracts only capsules
        # 2k / 2k+1 as required.
        v_ps = ps_v.tile([128, CK, 4, 32], F32, tag="v")
        with nc.named_scope(f"r{r}_v"):
            for ck in range(CK):
                for k in range(NDK):
                    nc.tensor.matmul(
                        out=v_ps[:, ck, :, 2 * k:2 * k + 2],
                        lhsT=wt2[:, k, ck * 128:(ck + 1) * 128],
                        rhs=oT2[:, k, :].rearrange("p (b e) -> p b e", e=2),
                        start=True, stop=True,
                    )
            v_sb = work.tile([128, CK, 128], BF16, tag="vsb")
            nc.vector.tensor_copy(
                out=v_sb[:], in_=v_ps[:].rearrange("p c b n -> p c (b n)")
            )
        # logits[i, b, ic, n] then per-b softmax over n
        lg_ps = ps_lg.tile([128, BL, IC, N], F32, tag="lg")
        with nc.named_scope(f"r{r}_lg"):
            for b in range(BL):
                for ic in range(IC):
                    for ck in range(CK):
                        nc.tensor.matmul(
                            out=lg_ps[:, b, ic, :],
                            lhsT=ut[:, ck, b, ic * 128:(ic + 1) * 128],
                            rhs=v_sb[:, ck, 32 * b:32 * b + 32],
                            start=(ck == 0),
                            stop=(ck == CK - 1),
                        )
        with nc.named_scope(f"r{r}_sm"):
            for b in range(BL):
                e_sb = work.tile([128, IC, N], BF16, tag="e")
                nc.scalar.activation(out=e_sb[:], in_=lg_ps[:, b], func=AF.Exp)
                s_sb = work.tile([128, IC], F32, tag="s")
                nc.vector.reduce_sum(
                    out=s_sb[:], in_=e_sb[:], axis=mybir.AxisListType.X
                )
                sr_sb = work.tile([128, IC], F32, tag="sr")
                nc.vector.reciprocal(out=sr_sb[:], in_=s_sb[:])
                nc.vector.tensor_tensor(
                    csm[:, :, 32 * b:32 * b + 32],
                    e_sb[:],
                    sr_sb[:, :, None].to_broadcast([128, IC, N]),
                    MULT,
                )
        # T^T[c, ck, b, n] = sum_i u[i, c] csm[i, (b,n)]
        tt_ps = ps_t.tile([128, CK, BL, N], F32, tag="tt")
        with nc.named_scope(f"r{r}_t"):
            for b in range(BL):
                for ck in range(CK):
                    for ic in range(IC):
                        nc.tensor.matmul(
                            out=tt_ps[:, ck, b, :],
                            lhsT=u_bf[:, ic, b, ck * 128:(ck + 1) * 128],
                            rhs=csm[:, ic, 32 * b:32 * b + 32],
                            start=(ic == 0),
                            stop=(ic == IC - 1),
                        )
            tt_sb = work.tile([128, CK, BL, N], BF16, tag="ttsb")
            nc.vector.tensor_copy(out=tt_sb[:], in_=tt_ps[:])
        pre_ps = emit_pre(r, tt_sb)
        emit_squash(r, pre_ps)

    # ---- write out: out[b, 2k+e, d] = oF[e*64+d, k, 2b+e] ----
    with nc.named_scope("out"):
        for e in range(2):
            nc.vector.tensor_copy(
                out=o_out[e * 64:(e + 1) * 64, :, :],
                in_=oF[e * 64:(e + 1) * 64, :, e::2].rearrange("p k b -> p b k"),
            )
        dst = out_ap.rearrange("b n d -> b (n d)").rearrange(
            "b (k p) -> p b k", p=128
        )
        nc.sync.dma_start(out=dst, in_=o_out[:])


def build_program():
    nc = bacc.Bacc("TRN2", target_bir_lowering=False, debug=False)
    u_ap = nc.dram_tensor("u", [BL, I, C], BF16, kind="ExternalInput").ap()
    w_ap = nc.dram_tensor("w", [C, ND], BF16, kind="ExternalInput").ap()
    out_ap = nc.dram_tensor("out", [BL, N, D], F32, kind="ExternalOutput").ap()
    with tile.TileContext(nc) as tc:
        with ExitStack() as ctx:
            _capsule_body(ctx, tc, out_ap, u_ap, w_ap)
    nc.compile()
    return nc


def make_in_maps(u32: np.ndarray, w32: np.ndarray):
    u = np.ascontiguousarray(u32.astype(ml_dtypes.bfloat16))
    w = np.ascontiguousarray(w32.astype(ml_dtypes.bfloat16))
    return [{"u": u[i * BL:(i + 1) * BL], "w": w} for i in range(NCORES)]


_NC = None


def kernel(u_vecs: np.ndarray, W: np.ndarray) -> np.ndarray:
    global _NC
    u = np.asarray(u_vecs, dtype=np.float32)
    w = np.asarray(W, dtype=np.float32)
    assert u.shape == (B, I, C) and w.shape == (C, ND)
    if _NC is None:
        _NC = build_program()
    res = run_bass_kernel_spmd(_NC, make_in_maps(u, w), list(range(NCORES)))
    return np.concatenate(
        [res.results[i]["out"] for i in range(NCORES)], axis=0
    )


# revision 15
# speedup vs baseline: 1.2997x; 1.1389x over previous
"""Trainium2 Bass kernel for capsule routing (nn_Capsule).

Reference computation:
    u_hat = einsum('bic,ce->bie', u_vecs, W).reshape(B, I, N, D).transpose(0,2,1,3)
    b = 0
    for r in range(3):
        c = softmax(b, axis=1)                      # over capsules n
        out = squash(einsum('bni,bnid->bnd', c, u_hat))
        if r < 2: b = einsum('bnd,bnid->bni', out, u_hat)
    return out    # (B, N, D)

u_hat (32 MiB/core) is never materialized; every use factors through u and W:
    round 0:  c uniform = 1/N  ->  out0 = squash((1/N) * (sum_i u[b,i,:]) @ W)
    V[b,c,n]     = sum_d W[c,(n,d)] o[b,n,d]
    logits[b,i,n]= sum_c u[b,i,c] V[b,c,n]
    T[b,n,c]     = sum_i softmax(logits)[b,i,n] * u[b,i,c]
    pre[b,n,d]   = sum_c T[b,n,c] W[c,(n,d)]   -> out = squash(pre)

Implementation notes:
  - inputs are cast to bf16 on the host, halving HBM traffic and making every
    matmul stationary a full 128x128 bf16 tile (fast-weight-load path),
  - u^T comes from the DMA xbar transpose straight from DRAM (scalar queue)
    while u / W load on the sync and gpsimd queues — no PE setup transposes,
  - pre^T is computed per nd-chunk k (capsule pair 2k, 2k+1) with the 8
    matching (b, n-pair) columns of T^T moving; only the row-half matching a
    column's parity is valid, and the squash scale is built MASKED (zero on
    the invalid half) so o^T's garbage half reads as exact zeros,
  - that zero structure lets the next round's V contract o^T against W^T
    per nd-chunk directly: each (ck, k) matmul writes its own disjoint 8
    columns of one PSUM tile, no block-diagonal operand is materialized,
  - squash's partition-dim reductions over d use tiny block-ones matmuls
    ([128,2] / [2,128]) to sum and re-broadcast per 64-row half; rsqrt is a
    bit-trick seed + Newton steps on the Vector engine so the Scalar engine's
    activation table stays pinned on exp,
  - a burst of dummy matmuls on a constant tile during the DMA window keeps
    the PE clock ramped before the real work arrives.

Sharding: data-parallel over batch, 4 batches per core x 8 cores, W replicated.
"""

import numpy as np
from contextlib import ExitStack

import ml_dtypes

import concourse.bass as bass
import concourse.bacc as bacc
import concourse.tile as tile
from concourse import mybir
from concourse.bass_utils import run_bass_kernel_spmd
from concourse.masks import make_identity

B, I, C = 32, 1024, 256
N, D = 32, 64
ND = N * D
ROUTINGS = 3
EPS = 1e-7
NCORES = 8
BL = B // NCORES   # batches per core
IC = I // 128      # i chunks of 128
CK = C // 128      # c chunks of 128
NDK = ND // 128    # (n,d) chunks of 128
F32 = mybir.dt.float32
U32 = mybir.dt.uint32
BF16 = mybir.dt.bfloat16
MULT = mybir.AluOpType.mult
AF = mybir.ActivationFunctionType
RSQRT_MAGIC = 0x5F3759DF
WARMUP_MM = 40


def _capsule_body(ctx: ExitStack, tc: tile.TileContext, out_ap, u_ap, w_ap):
    nc = tc.nc
    ctx.enter_context(nc.allow_low_precision(reason="bf16 pipeline"))

    const = ctx.enter_context(tc.tile_pool(name="const", bufs=1))
    persist = ctx.enter_context(tc.tile_pool(name="persist", bufs=1))
    work = ctx.enter_context(tc.tile_pool(name="work", bufs=2))

    # ---- persistent SBUF tensors ----
    u_bf = persist.tile([128, IC, BL, C], BF16)   # [i, ic, b, c]
    ut = persist.tile([128, CK, BL, I], BF16)     # [c, ck, b, i]
    w_sb = persist.tile([128, CK, ND], BF16)      # [c, ck, (n,d)]
    wt2 = persist.tile([128, NDK, C], BF16)       # [(n,d), k, c]
    csm = persist.tile([128, IC, 128], BF16)      # [i, ic, (b,n)]
    st = persist.tile([128, CK, BL], F32)         # [c, ck, b] col sums of u
    st8 = persist.tile([128, CK, 8], BF16)        # st widened to (b, e) cols
    oT2 = persist.tile([128, NDK, 8], BF16)       # o^T, invalid half-zeros
    oF = persist.tile([128, NDK, 8], F32)         # final-round fp32 o^T
    o_out = persist.tile([128, BL, NDK], F32)     # DMA-friendly output layout

    # ---- input DMA (sync queue): W first (small, unblocks W-transposes),
    # then u batch by batch ----
    nc.sync.dma_start(
        out=w_sb[:], in_=w_ap.rearrange("(ck p) nd -> p ck nd", p=128)
    )
    u_r = u_ap.rearrange("b (ic p) c -> b p ic c", p=128)
    for b in range(BL):
        nc.sync.dma_start(out=u_bf[:, :, b, :], in_=u_r[b])

    # ---- constants (all partition-0/64-aligned writes) ----
    ident = const.tile([128, 128], F32)
    make_identity(nc, ident[:])
    blk1 = const.tile([128, 2], BF16)
    nc.gpsimd.memset(blk1[:], 0.0)
    nc.gpsimd.memset(blk1[0:64, 0:1], 1.0)
    nc.gpsimd.memset(blk1[64:128, 1:2], 1.0)
    blk1f = const.tile([128, 2], F32)
    nc.gpsimd.memset(blk1f[:], 0.0)
    nc.gpsimd.memset(blk1f[0:64, 0:1], 1.0)
    nc.gpsimd.memset(blk1f[64:128, 1:2], 1.0)
    # bmask[p, j] = 1/64 where j%2 == p//64 -> M8 = blk1f^T @ bmask [2, 8]
    bmask = const.tile([128, 8], F32)
    nc.gpsimd.memset(bmask[:], 0.0)
    nc.gpsimd.memset(bmask[0:64, 0::2], 1.0 / 64)
    nc.gpsimd.memset(bmask[64:128, 1::2], 1.0 / 64)
    magic = const.tile([2, 128], U32)
    nc.gpsimd.memset(magic[:], RSQRT_MAGIC)
    warm = const.tile([128, 128], BF16)
    nc.gpsimd.memset(warm[:], 0.001)
    ident_bf = const.tile([128, 128], BF16)
    make_identity(nc, ident_bf[:])

    blk1T = const.tile([2, 128], F32)
    blk1T_n = const.tile([2, 128], F32)
    m8 = const.tile([2, 8], F32)

    with tc.tile_pool(name="ps_setup", bufs=2, space="PSUM") as ps_setup:
        # blk1T[e, p] = (p//64 == e) via PE transpose; M8[e, j] = (j%2 == e)
        cps = ps_setup.tile([2, 136], F32, tag="cps")
        nc.tensor.transpose(cps[:, 0:128], blk1f[:], ident[:])
        nc.tensor.matmul(
            out=cps[:, 128:136], lhsT=blk1f[:], rhs=bmask[:], start=True, stop=True
        )
        nc.vector.tensor_copy(out=blk1T[:], in_=cps[:, 0:128])
        nc.vector.tensor_scalar(
            out=blk1T_n[:], in0=cps[:, 0:128], scalar1=1.0 / N, scalar2=None,
            op0=MULT,
        )
        nc.vector.tensor_copy(out=m8[:], in_=cps[:, 128:136])

        # preload the exp activation table while DMAs are in flight
        warm_act = work.tile([1, 2], F32, tag="wa")
        nc.scalar.activation(out=warm_act[:], in_=blk1f[0:1, 0:2], func=AF.Exp)

        # PE warmup: ramp the tensor-engine clock during the DMA window
        warm_ps = ps_setup.tile([128, 128], F32, tag="warm")
        with nc.named_scope("warmup"):
            for _ in range(WARMUP_MM):
                nc.tensor.matmul(
                    out=warm_ps[:], lhsT=warm[:], rhs=warm[:],
                    start=True, stop=True,
                )

        # wt2[(nd), k, c] from w_sb via matmul-transpose (w_chunk^T @ I)
        with nc.named_scope("wtr"):
            for g in range(8):
                wt_ps = ps_setup.tile([128, 4, 128], F32, tag="wtr")
                for t in range(4):
                    h, k = divmod(g * 4 + t, NDK)
                    nc.tensor.matmul(
                        out=wt_ps[:, t, :],
                        lhsT=w_sb[:, h, k * 128:(k + 1) * 128],
                        rhs=ident_bf[:],
                        start=True, stop=True,
                    )
                h, k0 = divmod(g * 4, NDK)
                dstv = wt2[:, k0:k0 + 4, h * 128:(h + 1) * 128]
                if g % 2 == 0:
                    nc.vector.tensor_copy(out=dstv, in_=wt_ps[:])
                else:
                    nc.scalar.copy(out=dstv, in_=wt_ps[:])

        # ut[c, ck, b, i] from u_bf via matmul-transpose, 4 i-chunks per
        # PSUM bank; copies rotate across Vector/Scalar/Pool
        with nc.named_scope("utr"):
            g = 0
            for b in range(BL):
                for ck in range(CK):
                    for half in range(2):
                        ut_ps = ps_setup.tile([128, 4, 128], F32, tag="utr")
                        for t in range(4):
                            ic = half * 4 + t
                            nc.tensor.matmul(
                                out=ut_ps[:, t, :],
                                lhsT=u_bf[:, ic, b, ck * 128:(ck + 1) * 128],
                                rhs=ident_bf[:],
                                start=True, stop=True,
                            )
                        dstv = ut[:, ck, b, half * 512:(half + 1) * 512]
                        dstv = dstv.rearrange("p (t i) -> p t i", t=4)
                        if g % 2 == 0:
                            nc.vector.tensor_copy(out=dstv, in_=ut_ps[:])
                        else:
                            nc.scalar.copy(out=dstv, in_=ut_ps[:])
                        g += 1

    ps_v = ctx.enter_context(tc.tile_pool(name="ps_v", bufs=1, space="PSUM"))
    ps_lg = ctx.enter_context(tc.tile_pool(name="ps_lg", bufs=1, space="PSUM"))
    ps_t = ctx.enter_context(tc.tile_pool(name="ps_t", bufs=1, space="PSUM"))
    ps_pre = ctx.enter_context(tc.tile_pool(name="ps_pre", bufs=1, space="PSUM"))
    ps_sq = ctx.enter_context(tc.tile_pool(name="ps_sq", bufs=1, space="PSUM"))

    # ---- st: column sums of u over i (from ut), split across DVE and ACT ----
    with nc.named_scope("st"):
        scratch = work.tile([128, I], BF16, tag="stscratch")
        for ck in range(CK):
            for b in range(BL):
                if b % 2 == 0:
                    nc.vector.reduce_sum(
                        out=st[:, ck, b:b + 1], in_=ut[:, ck, b, :],
                        axis=mybir.AxisListType.X,
                    )
                else:
                    nc.scalar.activation(
                        out=scratch[:], in_=ut[:, ck, b, :], func=AF.Copy,
                        accum_out=st[:, ck, b:b + 1],
                    )
        st8v = st8[:].rearrange("p h (b e) -> p h b e", e=2)
        for e in range(2):
            nc.vector.tensor_copy(out=st8v[:, :, :, e], in_=st[:])

    def emit_pre(r, tt_sb):
        """pre_ps[p, k, (b,e)] = sum_c W[c, k-chunk] . moving; valid rows of
        column j=(b,e) are the half p//64 == e."""
        pre_ps = ps_pre.tile([128, NDK, 8], F32, tag="pre")
        with nc.named_scope(f"r{r}_pre"):
            for k in range(NDK):
                for h in range(CK):
                    if r == 0:
                        rhs = st8[:, h, :]
                    else:
                        rhs = tt_sb[:, h, :, 2 * k:2 * k + 2]
                    nc.tensor.matmul(
                        out=pre_ps[:, k, :],
                        lhsT=w_sb[:, h, k * 128:(k + 1) * 128],
                        rhs=rhs,
                        start=(h == 0),
                        stop=(h == CK - 1),
                    )
        return pre_ps

    def emit_squash(r, pre_ps):
        """o = pre * rsqrt(blocksum(pre^2) * scale + eps), with the rsqrt
        broadcast MASKED so each column's invalid row-half becomes zero."""
        with nc.named_scope(f"r{r}_sq"):
            sq_sb = work.tile([128, NDK, 8], BF16, tag="sq")
            nc.scalar.activation(out=sq_sb[:], in_=pre_ps[:], func=AF.Square)
            pre_sb = work.tile([128, NDK, 8], F32, tag="presb")
            nc.vector.tensor_copy(out=pre_sb[:], in_=pre_ps[:])
            ss_ps = ps_sq.tile([2, 128], F32, tag="ss")
            nc.tensor.matmul(
                out=ss_ps[:], lhsT=blk1[:],
                rhs=sq_sb[:].rearrange("p a b -> p (a b)"),
                start=True, stop=True,
            )
            x_sb = work.tile([2, 128], F32, tag="x")
            nc.vector.tensor_scalar(
                out=x_sb[:], in0=ss_ps[:],
                scalar1=(1.0 / (N * N) if r == 0 else 1.0), scalar2=EPS,
                op0=MULT, op1=mybir.AluOpType.add,
            )
            # rsqrt: y0 = bitcast(magic - (bitcast(x) >> 1)); 2 Newton steps
            yb_sb = work.tile([2, 128], U32, tag="yb")
            nc.vector.tensor_scalar(
                out=yb_sb[:], in0=x_sb[:].bitcast(U32), scalar1=1, scalar2=None,
                op0=mybir.AluOpType.logical_shift_right,
            )
            y_sb = work.tile([2, 128], F32, tag="y")
            nc.vector.tensor_tensor(
                y_sb[:].bitcast(U32), magic[:], yb_sb[:],
                mybir.AluOpType.subtract,
            )
            for _ in range(2):
                t1 = work.tile([2, 128], F32, tag="nt1")
                nc.vector.tensor_mul(t1[:], y_sb[:], y_sb[:])
                nc.vector.scalar_tensor_tensor(
                    out=t1[:], in0=t1[:], scalar=-0.5, in1=x_sb[:],
                    op0=MULT, op1=MULT,
                )
                y2 = work.tile([2, 128], F32, tag="y2")
                nc.vector.scalar_tensor_tensor(
                    out=y2[:], in0=t1[:], scalar=1.5, in1=y_sb[:],
                    op0=mybir.AluOpType.add, op1=MULT,
                )
                y_sb = y2
            # mask invalid (j-parity != row-half) entries to zero, then
            # re-broadcast per half: rsqE[p, (k,j)] = yM[p//64, (k,j)]
            ym = work.tile([2, NDK, 8], F32, tag="ym")
            nc.vector.tensor_tensor(
                ym[:], y_sb[:].rearrange("p (k j) -> p k j", j=8),
                m8[:, None, :].to_broadcast([2, NDK, 8]), MULT,
            )
            rsqE_ps = ps_sq.tile([128, 128], F32, tag="rsqE")
            nc.tensor.matmul(
                out=rsqE_ps[:], lhsT=(blk1T_n[:] if r == 0 else blk1T[:]),
                rhs=ym[:].rearrange("p a b -> p (a b)"), start=True, stop=True,
            )
            dst = oF if r == ROUTINGS - 1 else oT2
            nc.vector.tensor_tensor(
                dst[:].rearrange("p a b -> p (a b)"),
                pre_sb[:].rearrange("p a b -> p (a b)"),
                rsqE_ps[:], MULT,
            )

    # ---- round 0 ----
    pre_ps = emit_pre(0, None)
    emit_squash(0, pre_ps)

    for r in range(1, ROUTINGS):
        # V[c, ck, (b,n)]: per (ck, k) one matmul into disjoint 8 columns;
        # oT2's invalid halves are zero, so each k contracts only capsules
        # 2k / 2k+1 as required.
        v_ps = ps_v.tile([128, CK, 4, 32], F32, tag="v")
        with nc.named_scope(f"r{r}_v"):
            for ck in range(CK):
                for k in range(NDK):
                    nc.tensor.matmul(
                        out=v_ps[:, ck, :, 2 * k:2 * k + 2],
                        lhsT=wt2[:, k, ck * 128:(ck + 1) * 128],
                        rhs=oT2[:, k, :].rearrange("p (b e) -> p b e", e=2),
                        start=True, stop=True,
                    )
            v_sb = work.tile([128, CK, 128], BF16, tag="vsb")
            nc.vector.tensor_copy(
                out=v_sb[:], in_=v_ps[:].rearrange("p c b n -> p c (b n)")
            )
        # logits[i, b, ic, n] then per-b softmax over n
        lg_ps = ps_lg.tile([128, BL, IC, N], F32, tag="lg")
        with nc.named_scope(f"r{r}_lg"):
            for b in range(BL):
                for ic in range(IC):
                    for ck in range(CK):
                        nc.tensor.matmul(
                            out=lg_ps[:, b, ic, :],
                            lhsT=ut[:, ck, b, ic * 128:(ic + 1) * 128],
                            rhs=v_sb[:, ck, 32 * b:32 * b + 32],
                            start=(ck == 0),
                            stop=(ck == CK - 1),
                        )
        with nc.named_scope(f"r{r}_sm"):
            for b in range(BL):
                e_sb = work.tile([128, IC, N], BF16, tag="e")
                nc.scalar.activation(out=e_sb[:], in_=lg_ps[:, b], func=AF.Exp)
                s_sb = work.tile([128, IC], F32, tag="s")
                nc.vector.reduce_sum(
                    out=s_sb[:], in_=e_sb[:], axis=mybir.AxisListType.X
                )
                sr_sb = work.tile([128, IC], F32, tag="sr")
                nc.vector.reciprocal(out=sr_sb[:], in_=s_sb[:])
                nc.vector.tensor_tensor(
                    csm[:, :, 32 * b:32 * b + 32],
                    e_sb[:],
                    sr_sb[:, :, None].to_broadcast([128, IC, N]),
                    MULT,
                )
        # T^T[c, ck, b, n] = sum_i u[i, c] csm[i, (b,n)]
        tt_ps = ps_t.tile([128, CK, BL, N], F32, tag="tt")
        with nc.named_scope(f"r{r}_t"):
            for b in range(BL):
                for ck in range(CK):
                    for ic in range(IC):
                        nc.tensor.matmul(
                            out=tt_ps[:, ck, b, :],
                            lhsT=u_bf[:, ic, b, ck * 128:(ck + 1) * 128],
                            rhs=csm[:, ic, 32 * b:32 * b + 32],
                            start=(ic == 0),
                            stop=(ic == IC - 1),
                        )
            tt_sb = work.tile([128, CK, BL, N], BF16, tag="ttsb")
            nc.vector.tensor_copy(out=tt_sb[:], in_=tt_ps[:])
        pre_ps = emit_pre(r, tt_sb)
        emit_squash(r, pre_ps)

    # ---- write out: out[b, 2k+e, d] = oF[e*64+d, k, 2b+e] ----
    with nc.named_scope("out"):
        for e in range(2):
            nc.vector.tensor_copy(
                out=o_out[e * 64:(e + 1) * 64, :, :],
                in_=oF[e * 64:(e + 1) * 64, :, e::2].rearrange("p k b -> p b k"),
            )
        dst = out_ap.rearrange("b n d -> b (n d)").rearrange(
            "b (k p) -> p b k", p=128
        )
        nc.sync.dma_start(out=dst, in_=o_out[:])


def build_program():
    nc = bacc.Bacc("TRN2", target_bir_lowering=False, debug=False)
    u_ap = nc.dram_tensor("u", [BL, I, C], BF16, kind="ExternalInput").ap()
    w_ap = nc.dram_tensor("w", [C, ND], BF16, kind="ExternalInput").ap()
    out_ap = nc.dram_tensor("out", [BL, N, D], F32, kind="ExternalOutput").ap()
    with tile.TileContext(nc) as tc:
        with ExitStack() as ctx:
            _capsule_body(ctx, tc, out_ap, u_ap, w_ap)
    nc.compile()
    return nc


def make_in_maps(u32: np.ndarray, w32: np.ndarray):
    u = np.ascontiguousarray(u32.astype(ml_dtypes.bfloat16))
    w = np.ascontiguousarray(w32.astype(ml_dtypes.bfloat16))
    return [{"u": u[i * BL:(i + 1) * BL], "w": w} for i in range(NCORES)]


_NC = None


def kernel(u_vecs: np.ndarray, W: np.ndarray) -> np.ndarray:
    global _NC
    u = np.asarray(u_vecs, dtype=np.float32)
    w = np.asarray(W, dtype=np.float32)
    assert u.shape == (B, I, C) and w.shape == (C, ND)
    if _NC is None:
        _NC = build_program()
    res = run_bass_kernel_spmd(_NC, make_in_maps(u, w), list(range(NCORES)))
    return np.concatenate(
        [res.results[i]["out"] for i in range(NCORES)], axis=0
    )
